# revision 21
# baseline (speedup 1.0000x reference)
"""Trainium2 Bass kernel for PatchConv2d (3x3 conv + positional rank-1 correction + bias).

out[b,o,h,w] = sum_{i,dy,dx} x_pad[b,i,h+dy,w+dx] * K[o,i,dy,dx]
             + 0.1*(h+w) * patch_sum[b,h,w] + bias[o]
where patch_sum = 3x3 box sum of the channel-summed padded input.

Strategy (per core; data-parallel over batch, 2 batches/core):
  - The 9 conv taps are packed into 5 K=128-ish matmul passes over output
    pixels: three passes pair taps (d,0)+(d,1) via a dual-copy SBUF tile
    whose upper 64 partitions hold the input shifted by 1 element; one pass
    pairs (0,2)+(1,2) via a tile shifted by 130 elements (one padded row);
    the final pass does tap (2,2) with K=66, whose extra two contraction
    rows inject bias (ones row x bias weights) and the positional
    correction c[p] (c row x ones weights) directly into PSUM.
  - patch_sum: channel sums s come from one-hot ones-matmuls producing a
    [rows, 130] PSUM layout; w-direction box sum on DVE; h-direction box
    sum via a tiny tridiagonal band matmul; multiply by the 0.1*(h+w) grid
    on DVE; flattened into the c row by an SBUF->SBUF DMA.
  - Matmuls run in float32r (full fp32 data, full-rate PE streaming).
"""

import numpy as np

import concourse.bass as bass
import concourse.mybir as mybir
import concourse.tile as tile
from concourse import bacc
from concourse.bass_utils import run_bass_kernel_spmd

N_CORES = 8
B_PER_CORE = 2          # 16 batches / 8 cores
CI = 64                 # in channels
CO = 128                # out channels
H = W = 128             # output spatial dims
HP = WP = 130           # padded spatial dims
FLAT = HP * WP          # 16900
XPAD_LEN = 17040        # padded flat length (tail zeros for shifted reads)
CHUNK_ROWS = 32         # output rows per chunk
N_CHUNKS = H // CHUNK_ROWS          # 4
HALO_ROWS = CHUNK_ROWS + 2          # 34 input rows per chunk
CHUNK_W = HALO_ROWS * WP + 4        # 4424 sbuf tile width (loads 34 rows + shift tail)
PAIR2_W = 4164                      # covers max AP offset for the (0,2)/(1,2) pass
ROWS_PER_PSUM = 8                   # output rows per [128,1024] psum tile
TILES_PER_CHUNK = CHUNK_ROWS // ROWS_PER_PSUM  # 4

MM_DT = mybir.dt.float32r           # matmul data dtype (fp32 bits, fast PE mode)
F32 = mybir.dt.float32

_CACHE = {}


def _build_nc():
    nc = bacc.Bacc("TRN2", target_bir_lowering=False, debug=False)

    xp = nc.dram_tensor("xp", [B_PER_CORE, CI, XPAD_LEN], MM_DT, kind="ExternalInput")
    w1 = nc.dram_tensor("w1", [3, 128, CO], MM_DT, kind="ExternalInput")
    w2 = nc.dram_tensor("w2", [128, CO], MM_DT, kind="ExternalInput")
    wl = nc.dram_tensor("wl", [66, CO], MM_DT, kind="ExternalInput")
    oh = nc.dram_tensor("oh", [CI, HALO_ROWS * HALO_ROWS], MM_DT, kind="ExternalInput")
    bandw = nc.dram_tensor("bandw", [HALO_ROWS, CHUNK_ROWS], MM_DT, kind="ExternalInput")
    hgrid = nc.dram_tensor("hgrid", [CHUNK_ROWS, N_CHUNKS * W], F32, kind="ExternalInput")
    onesrow = nc.dram_tensor("onesrow", [CHUNK_W], MM_DT, kind="ExternalInput")
    out = nc.dram_tensor("out", [B_PER_CORE, CO, H, W], F32, kind="ExternalOutput")

    with tile.TileContext(nc) as tc:
        with (
            tc.tile_pool(name="consts", bufs=1) as cpool,
            tc.tile_pool(name="p1pool", bufs=4) as p1pool,
            tc.tile_pool(name="chunks", bufs=2) as kpool,
            tc.tile_pool(name="small", bufs=2) as spool,
            tc.tile_pool(name="ostg", bufs=3) as opool,
            tc.tile_pool(name="mpsum", bufs=4, space="PSUM") as mpsum,
            tc.tile_pool(name="spsum", bufs=2, space="PSUM") as spsum,
            tc.tile_pool(name="bpsum", bufs=2, space="PSUM") as bpsum,
        ):
            # ---- load constants once ----
            w1_ts = []
            for d in range(3):
                w1_d = cpool.tile([128, CO], MM_DT, tag=f"w1_{d}")
                nc.sync.dma_start(w1_d[:], w1[d])
                w1_ts.append(w1_d)
            w2_t = cpool.tile([128, CO], MM_DT, tag="w2")
            nc.sync.dma_start(w2_t[:], w2[:])
            wl_t = cpool.tile([66, CO], MM_DT, tag="wl")
            nc.sync.dma_start(wl_t[:], wl[:])
            oh_t = cpool.tile([CI, HALO_ROWS * HALO_ROWS], MM_DT, tag="oh")
            nc.sync.dma_start(oh_t[:], oh[:])
            band_t = cpool.tile([HALO_ROWS, CHUNK_ROWS], MM_DT, tag="band")
            nc.sync.dma_start(band_t[:], bandw[:])
            hg_t = cpool.tile([CHUNK_ROWS, N_CHUNKS * W], F32, tag="hg")
            nc.sync.dma_start(hg_t[:], hgrid[:])
            w1_aps = [w1_ts[d][:] for d in range(3)]

            SP = 2610      # prefix/suffix split for pair copies
            SPL = 2400     # split for the 'last' copy
            chunk_seq = [(b, k) for b in range(B_PER_CORE)
                         for k in range(N_CHUNKS)]
            pair1_tiles = {}

            def load_pair1(ci):
                bb, kk = chunk_seq[ci]
                ff = kk * CHUNK_ROWS * WP
                t = p1pool.tile([128, CHUNK_W], MM_DT, tag="pair1",
                                name=f"pair1_{bb}_{kk}")
                nc.gpsimd.dma_start(t[0:64, 0:SP + WP], xp[bb, :, ff:ff + SP + WP])
                nc.gpsimd.dma_start(t[0:64, SP + WP:CHUNK_W],
                                    xp[bb, :, ff + SP + WP:ff + CHUNK_W])
                pair1_tiles[ci] = t

            # software prefetch: keep 3 chunk loads in flight ahead of use
            load_pair1(0)
            load_pair1(1)
            load_pair1(2)
            for ci, (b, k) in enumerate(chunk_seq):
                    h0 = k * CHUNK_ROWS
                    f0 = h0 * WP
                    # ---- load + replicate input chunk ----
                    # Copies are split prefix/suffix and ordered by first-use
                    # time so early matmuls never wait on a full-width copy.
                    pair1 = pair1_tiles.pop(ci)
                    pair2 = kpool.tile([128, PAIR2_W], MM_DT, tag="pair2")
                    last = kpool.tile([66, CHUNK_W], MM_DT, tag="last")
                    # DVE: shift-by-1 upper half (pairs (d,0)+(d,1)), prefix
                    nc.vector.tensor_copy(pair1[64:128, 0:SP], pair1[0:64, 1:SP + 1])
                    # ACT: pair2 low = straight copy, prefix
                    nc.scalar.copy(pair2[0:64, 0:SP], pair1[0:64, 0:SP])
                    # DVE: shift-by-130 upper half (pair (0,2)+(1,2)), prefix
                    nc.vector.tensor_copy(pair2[64:128, 0:SP],
                                          pair1[0:64, WP:WP + SP])
                    nc.sync.dma_start(last[64:65, :], onesrow[None, :])

                    # ---- channel sums s for the 34 halo rows ----
                    s_ps = spsum.tile([HALO_ROWS, WP], F32, tag="sps")
                    for r in range(HALO_ROWS):
                        nc.tensor.matmul(
                            s_ps[:],
                            oh_t[:, r * HALO_ROWS:(r + 1) * HALO_ROWS],
                            pair1[0:64, r * WP:(r + 1) * WP],
                            start=(r == 0), stop=(r == HALO_ROWS - 1),
                        )
                    if ci + 3 < len(chunk_seq):
                        load_pair1(ci + 3)
                    s_sb = spool.tile([HALO_ROWS, WP], F32, tag="ssb")
                    nc.scalar.copy(s_sb[:], s_ps[:])
                    # w-direction box sum
                    t1 = spool.tile([HALO_ROWS, W], F32, tag="t1")
                    nc.vector.tensor_add(t1[:], s_sb[:, 0:W], s_sb[:, 1:W + 1])
                    t2r = spool.tile([HALO_ROWS, W], MM_DT, tag="t2r")
                    nc.vector.tensor_add(t2r[:], t1[:], s_sb[:, 2:W + 2])
                    # DVE: 'last' low copy prefix (needed by first mm5s)
                    nc.vector.tensor_copy(last[0:64, 0:SPL], pair1[0:64, 0:SPL])
                    # ACT: pair2 low suffix
                    nc.scalar.copy(pair2[0:64, SP:PAIR2_W], pair1[0:64, SP:PAIR2_W])
                    b_ps = bpsum.tile([CHUNK_ROWS, W], F32, tag="bps")
                    c_sb = spool.tile([CHUNK_ROWS, W], MM_DT, tag="csb")

                    def rhs(tile_ap, off):
                        return (tile_ap[:, off:off + 4 * WP]
                                .rearrange("p (r w) -> p r w", w=WP)
                                [:, :, 0:W])

                    # ---- main conv passes; mm5 delayed 3 halves so the
                    # correction row is ready, band matmul slotted early ----
                    DELAY = 3
                    halves = []   # (psl, ob, p, half)
                    ostgs = {}

                    def finish_half(idx):
                        psl, ob, p, half = halves[idx]
                        nc.tensor.matmul(psl[:], wl_t[:],
                                         rhs(last, ob + 2 * WP + 2),
                                         start=False, stop=True)
                        if p not in ostgs:
                            ostgs[p] = opool.tile([CO, ROWS_PER_PSUM * W], F32,
                                                  tag="ostg", name="ostg")
                        nc.scalar.copy(
                            ostgs[p][:, half * 512:(half + 1) * 512], psl[:])
                        if half == 1:
                            hrow = h0 + p * ROWS_PER_PSUM
                            nc.scalar.dma_start(
                                out[b, :, hrow:hrow + ROWS_PER_PSUM, :],
                                ostgs[p][:].rearrange("p (r w) -> p r w", w=W),
                            )

                    for i in range(8):
                        p, half = divmod(i, 2)
                        psl = mpsum.tile([CO, 512], F32, tag="mps")
                        ob = p * ROWS_PER_PSUM * WP + half * 4 * WP
                        halves.append((psl, ob, p, half))
                        nc.tensor.matmul(psl[:], w1_aps[0],
                                         rhs(pair1, ob + 0), start=True, stop=False)
                        nc.tensor.matmul(psl[:], w1_aps[1],
                                         rhs(pair1, ob + WP), start=False, stop=False)
                        nc.tensor.matmul(psl[:], w1_aps[2],
                                         rhs(pair1, ob + 2 * WP), start=False, stop=False)
                        nc.tensor.matmul(psl[:], w2_t[:],
                                         rhs(pair2, ob + 2), start=False, stop=False)
                        if i == 1:
                            # h-direction box sum via tridiagonal band matmul
                            nc.tensor.matmul(b_ps[:], band_t[:], t2r[:],
                                             start=True, stop=True)
                            # c = 0.1*(h+w) * patch_sum
                            nc.vector.tensor_mul(c_sb[:], b_ps[:],
                                                 hg_t[:, k * W:(k + 1) * W])
                            # flatten c into 'last' row 65 with tap-(2,2)
                            # geometry (sync queue: never blocks loads)
                            nc.sync.dma_start(
                                last[65:66, 262:262 + CHUNK_ROWS * WP]
                                .rearrange("p (r w) -> p r w", w=WP)[:, :, 0:W],
                                c_sb[:],
                            )
                        if i == 2:
                            # copy suffixes (first needed by half 4+)
                            nc.vector.tensor_copy(pair1[64:128, SP:CHUNK_W - 1],
                                                  pair1[0:64, SP + 1:CHUNK_W])
                            nc.vector.tensor_copy(pair2[64:128, SP:PAIR2_W],
                                                  pair1[0:64, WP + SP:WP + PAIR2_W])
                        if i == 3:
                            nc.vector.tensor_copy(last[0:64, SPL:4420],
                                                  pair1[0:64, SPL:4420])
                        if i >= DELAY:
                            finish_half(i - DELAY)
                    for idx in range(8 - DELAY, 8):
                        finish_half(idx)
    nc.compile()
    return nc


def _prep_consts(kern, bias):
    kern = np.asarray(kern, np.float32)
    bias = np.asarray(bias, np.float32)
    w1 = np.zeros((3, 128, CO), np.float32)
    for d in range(3):
        w1[d, 0:64] = kern[:, :, d, 0].T
        w1[d, 64:128] = kern[:, :, d, 1].T
    w2 = np.zeros((128, CO), np.float32)
    w2[0:64] = kern[:, :, 0, 2].T
    w2[64:128] = kern[:, :, 1, 2].T
    wl = np.zeros((66, CO), np.float32)
    wl[0:64] = kern[:, :, 2, 2].T
    wl[64] = bias
    wl[65] = 1.0
    oh = np.zeros((CI, HALO_ROWS * HALO_ROWS), np.float32)
    for r in range(HALO_ROWS):
        oh[:, r * HALO_ROWS + r] = 1.0
    band = np.zeros((HALO_ROWS, CHUNK_ROWS), np.float32)
    for m in range(CHUNK_ROWS):
        band[m:m + 3, m] = 1.0
    hgrid = np.zeros((CHUNK_ROWS, N_CHUNKS * W), np.float32)
    for k in range(N_CHUNKS):
        hh = k * CHUNK_ROWS + np.arange(CHUNK_ROWS)[:, None]
        ww = np.arange(W)[None, :]
        hgrid[:, k * W:(k + 1) * W] = 0.1 * (hh + ww)
    onesrow = np.ones((CHUNK_W,), np.float32)
    return dict(w1=w1, w2=w2, wl=wl, oh=oh, bandw=band, hgrid=hgrid,
                onesrow=onesrow)


def _prep_x(x):
    x = np.asarray(x, np.float32)
    n = x.shape[0]
    xpad = np.zeros((n, CI, HP, WP), np.float32)
    xpad[:, :, 1:129, 1:129] = x
    flat = xpad.reshape(n, CI, FLAT)
    xp = np.zeros((n, CI, XPAD_LEN), np.float32)
    xp[:, :, :FLAT] = flat
    return xp


def make_in_maps(x, kern, bias):
    consts = _prep_consts(kern, bias)
    xp = _prep_x(x)
    in_maps = []
    for c in range(N_CORES):
        m = dict(consts)
        m["xp"] = xp[c * B_PER_CORE:(c + 1) * B_PER_CORE]
        in_maps.append(m)
    return in_maps


def get_nc():
    if "nc" not in _CACHE:
        _CACHE["nc"] = _build_nc()
    return _CACHE["nc"]


def kernel(x, kernel, bias):
    nc = get_nc()
    in_maps = make_in_maps(x, kernel, bias)
    res = run_bass_kernel_spmd(nc, in_maps, core_ids=list(range(N_CORES)))
    outs = [res.results[c]["out"] for c in range(N_CORES)]
    return np.concatenate(outs, axis=0).astype(np.float32)


if __name__ == "__main__":
    nc = get_nc()
    print("build+compile OK")


# revision 22
# speedup vs baseline: 1.1541x; 1.1541x over previous
"""Trainium2 Bass kernel for PatchConv2d (3x3 conv + positional rank-1 correction + bias).

out[b,o,h,w] = sum_{i,dy,dx} x_pad[b,i,h+dy,w+dx] * K[o,i,dy,dx]
             + 0.1*(h+w) * patch_sum[b,h,w] + bias[o]
where patch_sum = 3x3 box sum of the channel-summed padded input.

Strategy (per core; data-parallel over batch, 2 batches/core):
  - The 9 conv taps are packed into 5 K=128-ish matmul passes over output
    pixels: three passes pair taps (d,0)+(d,1) via a dual-copy SBUF tile
    whose upper 64 partitions hold the input shifted by 1 element; one pass
    pairs (0,2)+(1,2) via a tile shifted by 130 elements (one padded row);
    the final pass does tap (2,2) with K=66, whose extra two contraction
    rows inject bias (ones row x bias weights) and the positional
    correction c[p] (c row x ones weights) directly into PSUM.
  - patch_sum: channel sums s come from one-hot ones-matmuls producing a
    [rows, 130] PSUM layout; w-direction box sum on DVE; h-direction box
    sum via a tiny tridiagonal band matmul; multiply by the 0.1*(h+w) grid
    on DVE; flattened into the c row by an SBUF->SBUF DMA.
  - Matmuls run in float32r (full fp32 data, full-rate PE streaming).
"""

import numpy as np

import concourse.bass as bass
import concourse.mybir as mybir
import concourse.tile as tile
from concourse import bacc
from concourse.bass_utils import run_bass_kernel_spmd

N_CORES = 8
B_PER_CORE = 2          # 16 batches / 8 cores
CI = 64                 # in channels
CO = 128                # out channels
H = W = 128             # output spatial dims
HP = WP = 130           # padded spatial dims
FLAT = HP * WP          # 16900
XPAD_LEN = 17040        # padded flat length (tail zeros for shifted reads)
CHUNK_ROWS = 32         # output rows per chunk
N_CHUNKS = H // CHUNK_ROWS          # 4
HALO_ROWS = CHUNK_ROWS + 2          # 34 input rows per chunk
CHUNK_W = HALO_ROWS * WP + 4        # 4424 sbuf tile width (loads 34 rows + shift tail)
PAIR2_W = 4164                      # covers max AP offset for the (0,2)/(1,2) pass
ROWS_PER_PSUM = 8                   # output rows per [128,1024] psum tile
TILES_PER_CHUNK = CHUNK_ROWS // ROWS_PER_PSUM  # 4

MM_DT = mybir.dt.float32r           # matmul data dtype (fp32 bits, fast PE mode)
F32 = mybir.dt.float32

_CACHE = {}


def _build_nc():
    nc = bacc.Bacc("TRN2", target_bir_lowering=False, debug=False)

    xp = nc.dram_tensor("xp", [B_PER_CORE, CI, XPAD_LEN], MM_DT, kind="ExternalInput")
    w1 = nc.dram_tensor("w1", [3, 128, CO], MM_DT, kind="ExternalInput")
    w2 = nc.dram_tensor("w2", [128, CO], MM_DT, kind="ExternalInput")
    wl = nc.dram_tensor("wl", [66, CO], MM_DT, kind="ExternalInput")
    oh = nc.dram_tensor("oh", [CI, HALO_ROWS * HALO_ROWS], MM_DT, kind="ExternalInput")
    bandw = nc.dram_tensor("bandw", [HALO_ROWS, CHUNK_ROWS], MM_DT, kind="ExternalInput")
    hgrid = nc.dram_tensor("hgrid", [CHUNK_ROWS, N_CHUNKS * W], F32, kind="ExternalInput")
    onesrow = nc.dram_tensor("onesrow", [CHUNK_W], MM_DT, kind="ExternalInput")
    out = nc.dram_tensor("out", [B_PER_CORE, CO, H, W], F32, kind="ExternalOutput")

    with tile.TileContext(nc) as tc:
        with (
            tc.tile_pool(name="consts", bufs=1) as cpool,
            tc.tile_pool(name="p1pool", bufs=4) as p1pool,
            tc.tile_pool(name="chunks", bufs=2) as kpool,
            tc.tile_pool(name="small", bufs=2) as spool,
            tc.tile_pool(name="ostg", bufs=3) as opool,
            tc.tile_pool(name="mpsum", bufs=4, space="PSUM") as mpsum,
            tc.tile_pool(name="spsum", bufs=2, space="PSUM") as spsum,
            tc.tile_pool(name="bpsum", bufs=2, space="PSUM") as bpsum,
        ):
            # ---- load constants once ----
            w1_ts = []
            for d in range(3):
                w1_d = cpool.tile([128, CO], MM_DT, tag=f"w1_{d}")
                nc.sync.dma_start(w1_d[:], w1[d])
                w1_ts.append(w1_d)
            w2_t = cpool.tile([128, CO], MM_DT, tag="w2")
            nc.sync.dma_start(w2_t[:], w2[:])
            wl_t = cpool.tile([66, CO], MM_DT, tag="wl")
            nc.sync.dma_start(wl_t[:], wl[:])
            oh_t = cpool.tile([CI, HALO_ROWS * HALO_ROWS], MM_DT, tag="oh")
            nc.sync.dma_start(oh_t[:], oh[:])
            band_t = cpool.tile([HALO_ROWS, CHUNK_ROWS], MM_DT, tag="band")
            nc.sync.dma_start(band_t[:], bandw[:])
            hg_t = cpool.tile([CHUNK_ROWS, N_CHUNKS * W], F32, tag="hg")
            nc.sync.dma_start(hg_t[:], hgrid[:])
            w1_aps = [w1_ts[d][:] for d in range(3)]

            SP = 2610      # prefix/suffix split for pair copies
            SPL = 2400     # split for the 'last' copy
            chunk_seq = [(b, k) for b in range(B_PER_CORE)
                         for k in range(N_CHUNKS)]
            pair1_tiles = {}

            def load_pair1(ci):
                bb, kk = chunk_seq[ci]
                ff = kk * CHUNK_ROWS * WP
                t = p1pool.tile([128, CHUNK_W], MM_DT, tag="pair1",
                                name=f"pair1_{bb}_{kk}")
                nc.sync.dma_start(t[0:64, 0:SP + WP], xp[bb, :, ff:ff + SP + WP])
                nc.sync.dma_start(t[0:64, SP + WP:CHUNK_W],
                                  xp[bb, :, ff + SP + WP:ff + CHUNK_W])
                pair1_tiles[ci] = t

            # software prefetch: keep 3 chunk loads in flight ahead of use
            load_pair1(0)
            load_pair1(1)
            load_pair1(2)
            for ci, (b, k) in enumerate(chunk_seq):
                    h0 = k * CHUNK_ROWS
                    f0 = h0 * WP
                    # ---- load + replicate input chunk ----
                    # Copies are split prefix/suffix and ordered by first-use
                    # time so early matmuls never wait on a full-width copy.
                    pair1 = pair1_tiles.pop(ci)
                    pair2 = kpool.tile([128, PAIR2_W], MM_DT, tag="pair2")
                    last = kpool.tile([66, CHUNK_W], MM_DT, tag="last")
                    # DVE: shift-by-1 upper half (pairs (d,0)+(d,1)), prefix
                    nc.vector.tensor_copy(pair1[64:128, 0:SP], pair1[0:64, 1:SP + 1])
                    # ACT: pair2 low = straight copy, prefix
                    nc.scalar.copy(pair2[0:64, 0:SP], pair1[0:64, 0:SP])
                    # DVE: shift-by-130 upper half (pair (0,2)+(1,2)), prefix
                    nc.vector.tensor_copy(pair2[64:128, 0:SP],
                                          pair1[0:64, WP:WP + SP])
                    nc.scalar.dma_start(last[64:65, :], onesrow[None, :])

                    # ---- channel sums s for the 34 halo rows ----
                    s_ps = spsum.tile([HALO_ROWS, WP], F32, tag="sps")
                    for r in range(HALO_ROWS):
                        nc.tensor.matmul(
                            s_ps[:],
                            oh_t[:, r * HALO_ROWS:(r + 1) * HALO_ROWS],
                            pair1[0:64, r * WP:(r + 1) * WP],
                            start=(r == 0), stop=(r == HALO_ROWS - 1),
                        )
                    if ci + 3 < len(chunk_seq):
                        load_pair1(ci + 3)
                    s_sb = spool.tile([HALO_ROWS, WP], F32, tag="ssb")
                    nc.scalar.copy(s_sb[:], s_ps[:])
                    # w-direction box sum
                    t1 = spool.tile([HALO_ROWS, W], F32, tag="t1")
                    nc.vector.tensor_add(t1[:], s_sb[:, 0:W], s_sb[:, 1:W + 1])
                    t2r = spool.tile([HALO_ROWS, W], MM_DT, tag="t2r")
                    nc.vector.tensor_add(t2r[:], t1[:], s_sb[:, 2:W + 2])
                    # DVE: 'last' low copy prefix (needed by first mm5s)
                    nc.vector.tensor_copy(last[0:64, 0:SPL], pair1[0:64, 0:SPL])
                    # ACT: pair2 low suffix
                    nc.scalar.copy(pair2[0:64, SP:PAIR2_W], pair1[0:64, SP:PAIR2_W])
                    b_ps = bpsum.tile([CHUNK_ROWS, W], F32, tag="bps")
                    c_sb = spool.tile([CHUNK_ROWS, W], MM_DT, tag="csb")

                    def rhs(tile_ap, off):
                        return (tile_ap[:, off:off + 4 * WP]
                                .rearrange("p (r w) -> p r w", w=WP)
                                [:, :, 0:W])

                    # ---- main conv passes; mm5 delayed 3 halves so the
                    # correction row is ready, band matmul slotted early ----
                    DELAY = 3
                    halves = []   # (psl, ob, p, half)
                    ostgs = {}

                    def finish_half(idx):
                        psl, ob, p, half = halves[idx]
                        nc.tensor.matmul(psl[:], wl_t[:],
                                         rhs(last, ob + 2 * WP + 2),
                                         start=False, stop=True)
                        if p not in ostgs:
                            ostgs[p] = opool.tile([CO, ROWS_PER_PSUM * W], F32,
                                                  tag="ostg", name="ostg")
                        nc.scalar.copy(
                            ostgs[p][:, half * 512:(half + 1) * 512], psl[:])
                        if half == 1:
                            hrow = h0 + p * ROWS_PER_PSUM
                            nc.gpsimd.dma_start(
                                out[b, :, hrow:hrow + ROWS_PER_PSUM, :],
                                ostgs[p][:].rearrange("p (r w) -> p r w", w=W),
                            )

                    for i in range(8):
                        p, half = divmod(i, 2)
                        psl = mpsum.tile([CO, 512], F32, tag="mps")
                        ob = p * ROWS_PER_PSUM * WP + half * 4 * WP
                        halves.append((psl, ob, p, half))
                        nc.tensor.matmul(psl[:], w1_aps[0],
                                         rhs(pair1, ob + 0), start=True, stop=False)
                        nc.tensor.matmul(psl[:], w1_aps[1],
                                         rhs(pair1, ob + WP), start=False, stop=False)
                        nc.tensor.matmul(psl[:], w1_aps[2],
                                         rhs(pair1, ob + 2 * WP), start=False, stop=False)
                        nc.tensor.matmul(psl[:], w2_t[:],
                                         rhs(pair2, ob + 2), start=False, stop=False)
                        if i == 1:
                            # h-direction box sum via tridiagonal band matmul
                            nc.tensor.matmul(b_ps[:], band_t[:], t2r[:],
                                             start=True, stop=True)
                            # c = 0.1*(h+w) * patch_sum
                            nc.vector.tensor_mul(c_sb[:], b_ps[:],
                                                 hg_t[:, k * W:(k + 1) * W])
                            # flatten c into 'last' row 65 with tap-(2,2)
                            # geometry (scalar queue: never blocks loads)
                            nc.scalar.dma_start(
                                last[65:66, 262:262 + CHUNK_ROWS * WP]
                                .rearrange("p (r w) -> p r w", w=WP)[:, :, 0:W],
                                c_sb[:],
                            )
                        if i == 2:
                            # copy suffixes (first needed by half 4+)
                            nc.vector.tensor_copy(pair1[64:128, SP:CHUNK_W - 1],
                                                  pair1[0:64, SP + 1:CHUNK_W])
                            nc.vector.tensor_copy(pair2[64:128, SP:PAIR2_W],
                                                  pair1[0:64, WP + SP:WP + PAIR2_W])
                        if i == 3:
                            nc.vector.tensor_copy(last[0:64, SPL:4420],
                                                  pair1[0:64, SPL:4420])
                        if i >= DELAY:
                            finish_half(i - DELAY)
                    for idx in range(8 - DELAY, 8):
                        finish_half(idx)
    nc.compile()
    return nc


def _prep_consts(kern, bias):
    kern = np.asarray(kern, np.float32)
    bias = np.asarray(bias, np.float32)
    w1 = np.zeros((3, 128, CO), np.float32)
    for d in range(3):
        w1[d, 0:64] = kern[:, :, d, 0].T
        w1[d, 64:128] = kern[:, :, d, 1].T
    w2 = np.zeros((128, CO), np.float32)
    w2[0:64] = kern[:, :, 0, 2].T
    w2[64:128] = kern[:, :, 1, 2].T
    wl = np.zeros((66, CO), np.float32)
    wl[0:64] = kern[:, :, 2, 2].T
    wl[64] = bias
    wl[65] = 1.0
    oh = np.zeros((CI, HALO_ROWS * HALO_ROWS), np.float32)
    for r in range(HALO_ROWS):
        oh[:, r * HALO_ROWS + r] = 1.0
    band = np.zeros((HALO_ROWS, CHUNK_ROWS), np.float32)
    for m in range(CHUNK_ROWS):
        band[m:m + 3, m] = 1.0
    hgrid = np.zeros((CHUNK_ROWS, N_CHUNKS * W), np.float32)
    for k in range(N_CHUNKS):
        hh = k * CHUNK_ROWS + np.arange(CHUNK_ROWS)[:, None]
        ww = np.arange(W)[None, :]
        hgrid[:, k * W:(k + 1) * W] = 0.1 * (hh + ww)
    onesrow = np.ones((CHUNK_W,), np.float32)
    return dict(w1=w1, w2=w2, wl=wl, oh=oh, bandw=band, hgrid=hgrid,
                onesrow=onesrow)


def _prep_x(x):
    x = np.asarray(x, np.float32)
    n = x.shape[0]
    xpad = np.zeros((n, CI, HP, WP), np.float32)
    xpad[:, :, 1:129, 1:129] = x
    flat = xpad.reshape(n, CI, FLAT)
    xp = np.zeros((n, CI, XPAD_LEN), np.float32)
    xp[:, :, :FLAT] = flat
    return xp


def make_in_maps(x, kern, bias):
    consts = _prep_consts(kern, bias)
    xp = _prep_x(x)
    in_maps = []
    for c in range(N_CORES):
        m = dict(consts)
        m["xp"] = xp[c * B_PER_CORE:(c + 1) * B_PER_CORE]
        in_maps.append(m)
    return in_maps


def get_nc():
    if "nc" not in _CACHE:
        _CACHE["nc"] = _build_nc()
    return _CACHE["nc"]


def kernel(x, kernel, bias):
    nc = get_nc()
    in_maps = make_in_maps(x, kernel, bias)
    res = run_bass_kernel_spmd(nc, in_maps, core_ids=list(range(N_CORES)))
    outs = [res.results[c]["out"] for c in range(N_CORES)]
    return np.concatenate(outs, axis=0).astype(np.float32)


if __name__ == "__main__":
    nc = get_nc()
    print("build+compile OK")
